# revision 21
# baseline (speedup 1.0000x reference)
"""Fused transformer-block kernel for TRN2, 8-way data parallel over batch.

v2: fp8 (e4m3) DoubleRow matmuls for all GEMMs (QKV / V / proj / MLP1 /
MLP2) at 2 fp8 K-rows per partition per cycle; bf16 attention core.

Layout strategy per core (128 sequences of 96 tokens = 12288 tokens):
  - Residual stream in N-layout [token_part, feature_free]; LN stats via
    bn_stats/bn_aggr (one DVE pass); LN applies write e4m3 directly and
    are transposed to T-layout on the PE (fp8 transposes).
  - Weights pre-scaled x256 into e4m3 on the host; the 1/256 dequant is
    folded into the psum-readout ops (activation scale / tensor_scalar).
  - Causal mask folded into the scores psum via an extra PE matmul
    (I.T @ M adds M[t,s]); exp -> bf16, row sums on Pool, one normalize.
  - Elementwise work spread across ACT / DVE / Pool; latency-tolerant
    psum->sbuf copies (LN1 transpose, V) ride on DMA rings.
"""

import sys

sys.path.insert(0, "/opt/trn_rl_repo")

from contextlib import ExitStack

import ml_dtypes
import numpy as np

import concourse.bass as bass  # noqa: F401  (registers AP types)
import concourse.tile as tile
from concourse import bacc, bass_utils, mybir

# Steer Exp to the ln+exp act-func set (set ids are positional indices
# into act_info.json, so edit MEMBERSHIP, never order): with Ln and Exp
# resolving to one table, only the Gelu<->Exp transition reloads tables.
try:
    import concourse.hw_specs as _hws

    _orig_gat = _hws.get_activation_tables.__wrapped__

    def _gat(module_arch):
        t = dict(_orig_gat(module_arch))
        exp_af = mybir.ActivationFunctionType.Exp
        for name in list(t.keys()):
            if name != "natural_log_exp_and_others" and exp_af in t[name]:
                t[name] = t[name] - {exp_af}
        return t

    import functools

    _hws.get_activation_tables = functools.cache(_gat)
    bacc.get_activation_tables = _hws.get_activation_tables
except Exception:
    pass

# Cache walrus-compiled NEFFs on disk keyed by BIR hash.
try:
    import hashlib
    import os as _os
    import shutil as _shutil

    import concourse.bass2jax as _b2j

    _orig_cbk = _b2j.compile_bir_kernel

    def _cached_cbk(bir_json, tmpdir, neff_name="file.neff"):
        try:
            raw = bir_json if isinstance(bir_json, bytes) else bir_json.encode()
            h = hashlib.sha256(raw).hexdigest()[:24]
            cdir = "/tmp/neff_cache"
            _os.makedirs(cdir, exist_ok=True)
            cpath = _os.path.join(cdir, h + ".neff")
            if _os.path.exists(cpath):
                return cpath
        except Exception:
            return _orig_cbk(bir_json, tmpdir, neff_name)
        p = _orig_cbk(bir_json, tmpdir, neff_name)
        try:
            _shutil.copy(p, cpath)
        except Exception:
            pass
        return p

    if _orig_cbk.__name__ != "_cached_cbk":
        _b2j.compile_bir_kernel = _cached_cbk
except Exception:
    pass

B, T, C = 1024, 96, 512
H, D = 4, 128
F = 4 * C
EPS = 1e-5
SCALE = D**-0.5
WS = 256.0  # fp8 weight pre-scale
DQ = 1.0 / WS
MASKVAL = -1e4  # raw-score offset for masked (s > t) entries

NCORES = 8
SEQ_PER_CORE = B // NCORES  # 128
S = SEQ_PER_CORE * T  # 12288 tokens per core
NB = 4  # sequences per block
TOK = NB * T  # 384 tokens per block
NBLK = SEQ_PER_CORE // NB  # 32 blocks
TCH = TOK // 128  # 3 token chunks per block
KC = C // 128  # 4 feature chunks of C
FM = F // 128  # 16 feature chunks of F
GELU_G = 1  # fm chunks per Gelu (pa/pb buffering preferred)

F32 = mybir.dt.float32
BF16 = mybir.dt.bfloat16
FP8 = mybir.dt.float8e4
U16 = mybir.dt.uint16
AF = mybir.ActivationFunctionType
OP = mybir.AluOpType
DR = mybir.MatmulPerfMode.DoubleRow

# Per-stage W-residual correction (adds a second fp8 matmul per pair).
CORR = dict(qkv=False, v=False, proj=False, mlp1=False, mlp2=False)


def build(nblk=NBLK, corr=None, has_bq=False, has_bk=False, has_bv=False,
          has_bp=False, has_b1=False, has_b2=False):
    corr = dict(CORR) if corr is None else corr
    nc = bacc.Bacc("TRN2", target_bir_lowering=False, debug=False)

    def din(name, shape, dt):
        return nc.dram_tensor(name, shape, dt, kind="ExternalInput").ap()

    x_d = din("x", [S, C], F32)
    wq_d = din("wq", [C, C], FP8)
    wk_d = din("wk", [C, C], FP8)
    wv_d = din("wv", [C, C], FP8)
    wp_d = din("wp", [C, C], FP8)
    w1_d = din("w1", [C, F], FP8)
    w2_d = din("w2", [F, C], FP8)
    rq_d = din("rq", [C, C], FP8) if corr["qkv"] else None
    rk_d = din("rk", [C, C], FP8) if corr["qkv"] else None
    rv_d = din("rv", [C, C], FP8) if corr["v"] else None
    rp_d = din("rp", [C, C], FP8) if corr["proj"] else None
    r1_d = din("r1", [C, F], FP8) if corr["mlp1"] else None
    r2_d = din("r2", [F, C], FP8) if corr["mlp2"] else None
    maskm_d = din("maskm", [T, T], BF16)
    ident_d = din("ident", [128, 128], BF16)
    ident8_d = din("ident8", [128, 128], FP8)
    bq_d = din("bq", [C], F32) if has_bq else None
    bk_d = din("bk", [C], F32) if has_bk else None
    bv_d = din("bv_b", [T, C], F32) if has_bv else None
    bp_d = din("bp_b", [128, C], F32) if has_bp else None
    b1_d = din("b1", [F], F32) if has_b1 else None
    b2_d = din("b2_b", [128, C], F32) if has_b2 else None
    y_d = nc.dram_tensor("y", [S, C], F32, kind="ExternalOutput").ap()

    with tile.TileContext(nc) as tc, ExitStack() as ctx:
        wp_pool = ctx.enter_context(tc.tile_pool(name="wpool", bufs=1))
        ap_ = ctx.enter_context(tc.tile_pool(name="act", bufs=2))
        st = ctx.enter_context(tc.tile_pool(name="stat", bufs=3))
        hp = ctx.enter_context(tc.tile_pool(name="ht", bufs=1))
        ps = ctx.enter_context(tc.tile_pool(name="psum", bufs=1, space="PSUM"))

        # ---- resident weights ----
        def wload(name, d_ap, kchunks, fdim, dt=FP8, packed=False):
            t = wp_pool.tile([128, kchunks, fdim], dt, tag=name)
            pat = "(p kc) f -> p kc f" if packed else "(kc p) f -> p kc f"
            nc.sync.dma_start(t[:], d_ap.rearrange(pat, p=128))
            return t

        # wq/wk/wv/w1 are host-packed pair-interleaved (row = 256j+2p+i) to
        # match the uint16-xbar-transposed activations; wp/w2 stay chunked
        # (their stationary activations use chunked d/f layouts).
        wq_sb = wload("wq", wq_d, KC, C, packed=True)
        wk_sb = wload("wk", wk_d, KC, C, packed=True)
        wv_sb = wload("wv", wv_d, KC, C, packed=True)
        wp_sb = wload("wp", wp_d, KC, C)
        w1_sb = wload("w1", w1_d, KC, F, packed=True)
        w2_sb = wload("w2", w2_d, FM, C)
        rq_sb = wload("rq", rq_d, KC, C, packed=True) if corr["qkv"] else None
        rk_sb = wload("rk", rk_d, KC, C, packed=True) if corr["qkv"] else None
        rv_sb = wload("rv", rv_d, KC, C, packed=True) if corr["v"] else None
        rp_sb = wload("rp", rp_d, KC, C) if corr["proj"] else None
        r1_sb = wload("r1", r1_d, KC, F, packed=True) if corr["mlp1"] else None
        r2_sb = wload("r2", r2_d, FM, C) if corr["mlp2"] else None

        maskm_sb = wp_pool.tile([T, T], BF16, tag="maskm")
        nc.sync.dma_start(maskm_sb[:], maskm_d)
        ident_sb = wp_pool.tile([128, 128], BF16, tag="ident")
        nc.sync.dma_start(ident_sb[:], ident_d)
        ident8_sb = wp_pool.tile([128, 128], FP8, tag="ident8")
        nc.sync.dma_start(ident8_sb[:], ident8_d)
        eps_sb = wp_pool.tile([128, 1], F32, tag="eps")
        nc.vector.memset(eps_sb[:], EPS)
        if has_bq:
            bq_sb = wp_pool.tile([128, H], F32, tag="bq")
            nc.sync.dma_start(bq_sb[:], bq_d.rearrange("(h d) -> d h", d=128))
        if has_bk:
            bk_sb = wp_pool.tile([128, H], F32, tag="bk")
            nc.sync.dma_start(bk_sb[:], bk_d.rearrange("(h d) -> d h", d=128))
        if has_bv:
            bv_sb = wp_pool.tile([T, C], F32, tag="bv")
            nc.sync.dma_start(bv_sb[:], bv_d)
        if has_bp:
            bp_sb = wp_pool.tile([128, C], F32, tag="bp")
            nc.sync.dma_start(bp_sb[:], bp_d)
        if has_b1:
            b1_sb = wp_pool.tile([128, FM], F32, tag="b1")
            nc.sync.dma_start(b1_sb[:], b1_d.rearrange("(fm p) -> p fm", p=128))
        if has_b2:
            b2_sb = wp_pool.tile([128, C], F32, tag="b2")
            nc.sync.dma_start(b2_sb[:], b2_d)

        NPAIR = KC // 2  # DoubleRow kc pairs

        # ---- helpers ----
        def ln_stats(src, pref):
            """src [128, TCH, C] f32 -> (rstd, nmr) each [128, TCH] f32."""
            bns = st.tile([128, TCH, 6], F32, tag=pref + "bns")
            mv = st.tile([128, TCH, 2], F32, tag=pref + "mv")
            for i in range(TCH):
                nc.vector.bn_stats(bns[:, i, :], src[:, i, :])
                nc.vector.bn_aggr(mv[:, i, :], bns[:, i, :])
            lnv = st.tile([128, TCH], F32, tag=pref + "lnv")
            nc.scalar.activation(lnv[:], mv[:, :, 1], AF.Ln,
                                 bias=eps_sb[:, 0:1])
            rstd = st.tile([128, TCH], F32, tag=pref + "rstd")
            nc.scalar.activation(rstd[:], lnv[:], AF.Exp, scale=-0.5)
            nmr = st.tile([128, TCH], F32, tag=pref + "nmr")
            nc.vector.scalar_tensor_tensor(nmr[:], mv[:, :, 0], -1.0, rstd[:],
                                           OP.mult, OP.mult)
            return rstd, nmr

        def ln_apply(src, rstd, nmr, pref):
            """normalize -> fp8 [128, TCH, C]."""
            xn8 = ap_.tile([128, TCH, C], FP8, tag=pref + "xn8", bufs=3)
            for i in range(TCH):
                eng = nc.gpsimd
                eng.tensor_scalar(xn8[:, i, :], src[:, i, :],
                                  rstd[:, i : i + 1], nmr[:, i : i + 1],
                                  OP.mult, OP.add)
            return xn8

        def transpose8(xn8, pref):
            """uint16-packed DMA-xbar transpose: fp8 [128, TCH, C] ->
            [128, KC//2, TCH*128] u16 holding fp8 pairs (c=256j+2p+i)."""
            xnTu = ap_.tile([128, KC // 2, TCH * 128], U16, tag=pref + "xnT", bufs=3)
            xn8u = xn8[:].bitcast(U16)  # [128, TCH, C//2]
            for j in range(KC // 2):
                for mc in range(TCH):
                    nc.sync.dma_start_transpose(
                        out=xnTu[:, j, mc * 128 : (mc + 1) * 128],
                        in_=xn8u[:, mc, 128 * j : 128 * (j + 1)])
            return xnTu

        def rhs_pair(xnTu, j, lo=0, n=TOK):
            """DoubleRow [K,2,N] AP from the packed-u16 T-layout tile."""
            return (xnTu[:, j, lo : lo + n].bitcast(FP8)
                    .rearrange("p (t two) -> p two t", two=2))

        def mm_pairs(psum, lhs_fn, rhs_fn, rcorr_lhs_fn=None, rcorr_rhs_fn=None,
                     npair=NPAIR):
            """DoubleRow accumulation over kc pairs (+ optional residual)."""
            ncorr = npair if rcorr_lhs_fn is not None else 0
            total = npair + ncorr
            idx = 0
            for j in range(npair):
                nc.tensor.matmul(psum, lhs_fn(j), rhs_fn(j), perf_mode=DR,
                                 start=(idx == 0), stop=(idx == total - 1))
                idx += 1
            for j in range(ncorr):
                nc.tensor.matmul(psum, rcorr_lhs_fn(j), rcorr_rhs_fn(j),
                                 perf_mode=DR, start=False,
                                 stop=(idx == total - 1))
                idx += 1

        # ---- block stages ----
        def stage_a1_load(blk):
            row0 = blk * TOK
            x_sb = ap_.tile([128, TCH, C], F32, tag="x", bufs=4)
            nc.sync.dma_start(
                x_sb[:],
                x_d[row0 : row0 + TOK, :].rearrange("(ch p) c -> p ch c", p=128))
            rstd, nmr = ln_stats(x_sb, "a")
            xn8 = ln_apply(x_sb, rstd, nmr, "a")
            return x_sb, xn8

        def stage_a1_x(xn8):
            return transpose8(xn8, "a")

        def stage_a2(blk, xnT):
            """QKV + scores/softmax."""
            qt = ap_.tile([128, H, TOK], BF16, tag="qt")
            kt = ap_.tile([128, H, TOK], BF16, tag="kt")
            for di, (dst, w_sb, r_sb, b_sb) in enumerate((
                    (qt, wq_sb, rq_sb if corr["qkv"] else None,
                     bq_sb if has_bq else None),
                    (kt, wk_sb, rk_sb if corr["qkv"] else None,
                     bk_sb if has_bk else None))):
                for h in range(H):
                    p = ps.tile([128, TOK], F32, tag="pa", bufs=4)
                    hs = slice(h * 128, (h + 1) * 128)
                    mm_pairs(
                        p[:],
                        lambda j: w_sb[:, 2 * j : 2 * j + 2, hs],
                        lambda j: rhs_pair(xnT, j),
                        (lambda j: r_sb[:, 2 * j : 2 * j + 2, hs])
                        if r_sb is not None else None,
                        (lambda j: rhs_pair(xnT, j))
                        if r_sb is not None else None)
                    bias = b_sb[:, h : h + 1] if b_sb is not None else 0.0
                    i = di * H + h
                    if i % 2 == 0:
                        nc.vector.tensor_scalar(dst[:, h, :], p[:], DQ, bias,
                                                OP.mult, OP.add)
                    else:
                        nc.scalar.activation(dst[:, h, :], p[:], AF.Identity,
                                             scale=DQ, bias=bias)

            # scores + mask -> exp -> row sums -> normalize
            ee = ap_.tile([T, H * NB, T], BF16, tag="ee")
            dsum = st.tile([T, H * NB], F32, tag="dsum")
            for h in range(H):
                p = ps.tile([T, NB, T], F32, tag="pa", bufs=4)
                for b in range(NB):
                    nc.tensor.matmul(p[:, b, :], qt[:, h, b * T : (b + 1) * T],
                                     kt[:, h, b * T : (b + 1) * T],
                                     start=True, stop=True)
                sl = slice(h * NB, (h + 1) * NB)
                nc.scalar.activation(ee[:, sl, :], p[:], AF.Exp, scale=SCALE)
                nc.vector.tensor_mul(
                    out=ee[:, sl, :], in0=ee[:, sl, :],
                    in1=maskm_sb[:].unsqueeze(1).to_broadcast([T, NB, T]))
                nc.vector.tensor_reduce(dsum[:, sl], ee[:, sl, :],
                                        axis=mybir.AxisListType.X, op=OP.add)
            rr = st.tile([T, H * NB], F32, tag="rr")
            nc.vector.reciprocal(rr[:], dsum[:])
            nc.gpsimd.tensor_mul(
                out=ee[:], in0=ee[:],
                in1=rr[:].unsqueeze(2).to_broadcast([T, H * NB, T]))

            # V in T-layout (weights stationary, like Q/K), then
            # PE-transpose per sequence to N-layout for attn @ V.
            vtT = ap_.tile([128, H, TOK], BF16, tag="vtT")
            for h in range(H):
                p = ps.tile([128, TOK], F32, tag="pa", bufs=4)
                hs = slice(h * 128, (h + 1) * 128)
                mm_pairs(
                    p[:],
                    lambda j: wv_sb[:, 2 * j : 2 * j + 2, hs],
                    lambda j: rhs_pair(xnT, j),
                    (lambda j: rv_sb[:, 2 * j : 2 * j + 2, hs])
                    if corr["v"] else None,
                    (lambda j: rhs_pair(xnT, j)) if corr["v"] else None)
                if h % 2 == 0:
                    nc.scalar.activation(vtT[:, h, :], p[:], AF.Identity,
                                         scale=DQ)
                else:
                    nc.vector.tensor_scalar_mul(vtT[:, h, :], p[:], DQ)
            vt = ap_.tile([T, NB, C], BF16, tag="vt")
            for b in range(NB):
                pvt = ps.tile([T, H, 128], BF16, tag="pa", bufs=4)
                for h in range(H):
                    nc.tensor.transpose(pvt[:, h, :],
                                        vtT[:, h, b * T : (b + 1) * T],
                                        ident_sb[:])
                if has_bv:
                    nc.vector.tensor_add(out=vt[:, b, :], in0=pvt[:],
                                         in1=bv_sb[:])
                elif b % 2 == 0:
                    nc.scalar.activation(vt[:, b, :], pvt[:], AF.Identity)
                else:
                    nc.vector.tensor_copy(out=vt[:, b, :], in_=pvt[:])
            return vt, ee

        def stage_a2b(blk, vt, ee):
            """probs transpose + attn @ V -> ot (fp8 T-layout)."""
            pt = ee  # transposed probs overwrite ee in place
            for h in range(H):
                p = ps.tile([T, NB, T], BF16, tag="pa", bufs=4)
                for b in range(NB):
                    nc.tensor.transpose(p[:, b, :], ee[:, h * NB + b, :],
                                        ident_sb[:T, :T])
                if h % 2 == 0:
                    nc.scalar.activation(pt[:, h * NB : (h + 1) * NB, :], p[:],
                                         AF.Identity)
                else:
                    nc.vector.tensor_copy(
                        out=pt[:, h * NB : (h + 1) * NB, :], in_=p[:])
            ot = ap_.tile([128, H, TOK], FP8, tag="ot")
            for h in range(H):
                p = ps.tile([128, NB, T], F32, tag="pa", bufs=4)
                for b in range(NB):
                    nc.tensor.matmul(p[:, b, :], vt[:, b, h * 128 : (h + 1) * 128],
                                     pt[:, h * NB + b, :], start=True, stop=True)
                if h % 2 == 0:
                    nc.vector.tensor_copy(out=ot[:, h, :], in_=p[:])
                else:
                    nc.scalar.activation(ot[:, h, :], p[:], AF.Identity)
            return ot

        def stage_b1(blk, x_sb, ot):
            """proj + residual, LN2 stats/apply/transpose."""
            x2 = ap_.tile([128, TCH, C], F32, tag="x2")
            for mc in range(TCH):
                p = ps.tile([128, C], F32, tag="pb", bufs=4)
                ms = slice(mc * 128, (mc + 1) * 128)
                mm_pairs(
                    p[:],
                    lambda j: ot[:, 2 * j : 2 * j + 2, ms],
                    lambda j: wp_sb[:, 2 * j : 2 * j + 2, :],
                    (lambda j: ot[:, 2 * j : 2 * j + 2, ms])
                    if corr["proj"] else None,
                    (lambda j: rp_sb[:, 2 * j : 2 * j + 2, :])
                    if corr["proj"] else None)
                if has_bp:
                    nc.vector.tensor_add(out=p[:], in0=p[:], in1=bp_sb[:])
                nc.vector.scalar_tensor_tensor(x2[:, mc, :], p[:], DQ,
                                               x_sb[:, mc, :], OP.mult, OP.add)

            rstd2, nmr2 = ln_stats(x2, "b")
            xn2 = ln_apply(x2, rstd2, nmr2, "b")
            xn2T = transpose8(xn2, "b")
            return x2, xn2T

        def stage_b2(blk, x2, xn2T):
            """MLP + residual + store."""
            row0 = blk * TOK
            # MLP1 + batched Gelu -> h8
            ht = hp.tile([128, FM, TOK], FP8, tag="ht")
            for g in range(FM // GELU_G):
                p = ps.tile([128, GELU_G, 512], F32, tag="pb", bufs=4)
                for gi in range(GELU_G):
                    fm = g * GELU_G + gi
                    fs = slice(fm * 128, (fm + 1) * 128)
                    mm_pairs(
                        p[:, gi, :TOK],
                        lambda j: w1_sb[:, 2 * j : 2 * j + 2, fs],
                        lambda j: rhs_pair(xn2T, j),
                        (lambda j: r1_sb[:, 2 * j : 2 * j + 2, fs])
                        if corr["mlp1"] else None,
                        (lambda j: rhs_pair(xn2T, j))
                        if corr["mlp1"] else None)
                if has_b1:
                    for gi in range(GELU_G):
                        fm = g * GELU_G + gi
                        nc.scalar.activation(ht[:, fm, :], p[:, gi, :TOK],
                                             AF.Gelu, scale=DQ,
                                             bias=b1_sb[:, fm : fm + 1])
                else:
                    nc.scalar.activation(
                        ht[:, g * GELU_G : (g + 1) * GELU_G, :],
                        p[:, :, :TOK], AF.Gelu, scale=DQ)

            # MLP2 + residual -> store
            xo = ap_.tile([128, TCH, C], F32, tag="xo")
            for mc in range(TCH):
                p = ps.tile([128, C], F32, tag="pb", bufs=4)
                ms = slice(mc * 128, (mc + 1) * 128)
                mm_pairs(
                    p[:],
                    lambda j: ht[:, 2 * j : 2 * j + 2, ms],
                    lambda j: w2_sb[:, 2 * j : 2 * j + 2, :],
                    (lambda j: ht[:, 2 * j : 2 * j + 2, ms])
                    if corr["mlp2"] else None,
                    (lambda j: r2_sb[:, 2 * j : 2 * j + 2, :])
                    if corr["mlp2"] else None,
                    npair=FM // 2)
                if has_b2:
                    nc.vector.tensor_add(out=p[:], in0=p[:], in1=b2_sb[:])
                nc.vector.scalar_tensor_tensor(xo[:, mc, :], p[:], DQ,
                                               x2[:, mc, :], OP.mult, OP.add)
            nc.sync.dma_start(
                y_d[row0 : row0 + TOK, :].rearrange("(ch p) c -> p ch c", p=128),
                xo[:])

        # Software-pipelined emission (same skeleton as v1).
        xs, xn8s, xnTs, sm, ots = {}, {}, {}, {}, {}
        for b0 in range(min(3, nblk)):
            xs[b0], xn8s[b0] = stage_a1_load(b0)
            xnTs[b0] = stage_a1_x(xn8s.pop(b0))
        sm[0] = stage_a2(0, xnTs.pop(0))
        ots[0] = stage_a2b(0, *sm.pop(0))
        for blk in range(1, nblk):
            if blk + 2 < nblk:
                xs[blk + 2], xn8s[blk + 2] = stage_a1_load(blk + 2)
            sm[blk] = stage_a2(blk, xnTs.pop(blk))
            mid = stage_b1(blk - 1, xs.pop(blk - 1), ots.pop(blk - 1))
            ots[blk] = stage_a2b(blk, *sm.pop(blk))
            if blk + 2 in xn8s:
                xnTs[blk + 2] = stage_a1_x(xn8s.pop(blk + 2))
            stage_b2(blk - 1, *mid)
        mid = stage_b1(nblk - 1, xs.pop(nblk - 1), ots.pop(nblk - 1))
        stage_b2(nblk - 1, *mid)

    nc.compile()
    return nc


def fold(inputs):
    """Host-side folding: LN affines into weights, fp8 hi/residual split."""
    f32 = np.float32
    E4 = ml_dtypes.float8_e4m3
    g1 = np.asarray(inputs["g1"], f32)
    be1 = np.asarray(inputs["be1"], f32)
    g2 = np.asarray(inputs["g2"], f32)
    be2 = np.asarray(inputs["be2"], f32)

    def headcat(w):  # [H, C, D] -> [C, H*D]
        return np.concatenate([w[h] for h in range(H)], axis=1)

    wq = headcat(np.asarray(inputs["wq"], f32))
    wk = headcat(np.asarray(inputs["wk"], f32))
    wv = headcat(np.asarray(inputs["wv"], f32))
    wp_ = np.asarray(inputs["w_proj"], f32)
    w1 = np.asarray(inputs["w1"], f32)
    w2 = np.asarray(inputs["w2"], f32)

    wq_f = g1[:, None] * wq
    wk_f = g1[:, None] * wk
    wv_f = g1[:, None] * wv
    bq = be1 @ wq
    bk = be1 @ wk
    bv = be1 @ wv
    bp = np.asarray(inputs["b_proj"], f32)
    w1_f = g2[:, None] * w1
    b1 = np.asarray(inputs["b1"], f32) + be2 @ w1
    b2 = np.asarray(inputs["b2"], f32)

    def q8(w):
        hi = np.asarray(w * WS, E4)
        r = np.asarray(w * WS - hi.astype(f32), E4)
        return hi, r

    def pack_pairs(w):
        """Row permute [C, M]: packed[p*R + 2j+i] = w[256j + 2p + i]."""
        cd = w.shape[0]
        return (w.reshape(cd // 256, 128, 2, -1).transpose(1, 0, 2, 3)
                .reshape(cd, -1))

    wq8, rq8 = q8(wq_f)
    wk8, rk8 = q8(wk_f)
    wv8, rv8 = q8(wv_f)
    wp8, rp8 = q8(wp_)
    w18, r18 = q8(w1_f)
    w28, r28 = q8(w2)
    wq8, rq8 = pack_pairs(wq8), pack_pairs(rq8)
    wk8, rk8 = pack_pairs(wk8), pack_pairs(rk8)
    wv8, rv8 = pack_pairs(wv8), pack_pairs(rv8)
    w18, r18 = pack_pairs(w18), pack_pairs(r18)

    maskm = np.tril(np.ones((T, T), np.float32)).astype(ml_dtypes.bfloat16)
    ident = np.eye(128, dtype=ml_dtypes.bfloat16)
    ident8 = np.eye(128, dtype=E4)

    staged = {
        "wq": wq8, "wk": wk8, "wv": wv8, "wp": wp8, "w1": w18, "w2": w28,
        "maskm": maskm, "ident": ident, "ident8": ident8,
    }
    if CORR["qkv"]:
        staged["rq"] = rq8
        staged["rk"] = rk8
    if CORR["v"]:
        staged["rv"] = rv8
    if CORR["proj"]:
        staged["rp"] = rp8
    if CORR["mlp1"]:
        staged["r1"] = r18
    if CORR["mlp2"]:
        staged["r2"] = r28
    flags = {
        "has_bq": bool(np.any(bq)),
        "has_bk": bool(np.any(bk)),
        "has_bv": bool(np.any(bv)),
        "has_bp": bool(np.any(bp)),
        "has_b1": bool(np.any(b1)),
        "has_b2": bool(np.any(b2)),
    }
    if flags["has_bq"]:
        staged["bq"] = bq
    if flags["has_bk"]:
        staged["bk"] = bk
    if flags["has_bv"]:
        staged["bv_b"] = np.broadcast_to(bv, (T, C)).copy()
    if flags["has_bp"]:
        staged["bp_b"] = np.broadcast_to(bp, (128, C)).copy()
    if flags["has_b1"]:
        staged["b1"] = b1
    if flags["has_b2"]:
        staged["b2_b"] = np.broadcast_to(b2, (128, C)).copy()
    return staged, flags


_CACHE = {}


def kernel(**inputs):
    inputs = {k: np.asarray(v) for k, v in inputs.items()}
    staged, flags = fold(inputs)
    key = tuple(sorted(flags.items()))
    if key not in _CACHE:
        _CACHE[key] = build(**flags)
    nc = _CACHE[key]

    x = np.asarray(inputs["x"], np.float32).reshape(B, T * C)
    in_maps = []
    for c in range(NCORES):
        m = dict(staged)
        m["x"] = x[c * SEQ_PER_CORE : (c + 1) * SEQ_PER_CORE].reshape(S, C)
        in_maps.append(m)

    res = bass_utils.run_bass_kernel_spmd(nc, in_maps, core_ids=list(range(NCORES)))
    out = np.concatenate([r["y"] for r in res.results], axis=0)
    return out.reshape(B, T, C).astype(np.float32)
